# revision 1
# baseline (speedup 1.0000x reference)
"""v4: row-pair table in DRAM halves the gather call count vs v3.

Build once per core: U2[r, v] = [T[1023+r, 1023+v, :], T[1024+r, 1023+v, :]]
(6 floats per entry, 1024 rows x 1025 cols).  A sample's full 2x2 quad is
then 12 CONTIGUOUS floats at entry (u0-1023, v0-1023), fetched by a single
[128,1]-index indirect DMA per column (the HW-proven config).
"""

import os
import numpy as np

P = 128
N_CORES = 8
N = 4194304
NPC = N // N_CORES
W = 2048
K = 128
U2_ROWS = 1024
U2_COLS = 1025

_cached = {}


def _build(npc=NPC, k=K, n_cores=N_CORES):
    import concourse.bass as bass
    import concourse.tile as tile
    from concourse import bacc, mybir
    from concourse.tile import add_dep_helper
    from contextlib import ExitStack

    f32 = mybir.dt.float32
    i32 = mybir.dt.int32
    Alu = mybir.AluOpType
    K = k
    n_chunks = npc // (P * K)
    assert n_chunks * P * K == npc

    nc = bacc.Bacc(
        "TRN2",
        target_bir_lowering=False,
        debug=False,
        enable_asserts=False,
        num_devices=n_cores,
    )
    tex_t = nc.dram_tensor("texture", [W * W, 3], f32, kind="ExternalInput")
    uv_t = nc.dram_tensor("uvs", [npc, 2], f32, kind="ExternalInput")
    col_t = nc.dram_tensor("colors", [npc, 3], f32, kind="ExternalOutput")
    u2_t = nc.dram_tensor("u2tab", [U2_ROWS * U2_COLS, 6], f32, kind="Internal")

    tex_rows = tex_t.ap().rearrange("(r c) s -> r (c s)", c=W)   # [2048, 6144]
    u2_rows = u2_t.ap().rearrange("(r v) s -> r (v s)", v=U2_COLS)  # [1024, 6150]
    uv_view = uv_t.ap().rearrange("(c p k) two -> c p (k two)", p=P, k=K)
    col_view = col_t.ap().rearrange("(c p k) three -> c p (k three)", p=P, k=K)

    with tile.TileContext(nc) as tc:
        with ExitStack() as ctx:
            b_pool = ctx.enter_context(tc.tile_pool(name="build", bufs=2))
            uv_pool = ctx.enter_context(tc.tile_pool(name="uv", bufs=3))
            w_pool = ctx.enter_context(tc.tile_pool(name="work", bufs=2))
            q_pool = ctx.enter_context(tc.tile_pool(name="quads", bufs=3))
            o_pool = ctx.enter_context(tc.tile_pool(name="outs", bufs=3))

            # ---- build U2 (8 blocks of 128 rows) ----
            last_store = None
            for b in range(U2_ROWS // P):
                r0 = b * P
                ta = b_pool.tile([P, 3 * U2_COLS], f32)
                nc.sync.dma_start(
                    ta[:], tex_rows[1023 + r0 : 1023 + r0 + P, 3069 : 3069 + 3 * U2_COLS]
                )
                tb = b_pool.tile([P, 3 * U2_COLS], f32)
                nc.sync.dma_start(
                    tb[:], tex_rows[1024 + r0 : 1024 + r0 + P, 3069 : 3069 + 3 * U2_COLS]
                )
                ub = b_pool.tile([P, U2_COLS, 6], f32)
                nc.vector.tensor_copy(
                    ub[:, :, 0:3], ta[:].rearrange("p (v s) -> p v s", s=3)
                )
                nc.vector.tensor_copy(
                    ub[:, :, 3:6], tb[:].rearrange("p (v s) -> p v s", s=3)
                )
                last_store = nc.sync.dma_start(
                    u2_rows[r0 : r0 + P, :], ub[:].rearrange("p v s -> p (v s)")
                )

            # ---- sample pipeline ----
            for c in range(n_chunks):
                uv = uv_pool.tile([P, K, 2], f32)
                nc.sync.dma_start(uv[:].rearrange("p k two -> p (k two)"), uv_view[c])

                pv = w_pool.tile([P, K, 2], f32)
                nc.vector.tensor_scalar(pv[:], uv[:], 1.0, 1023.5, Alu.add, Alu.mult)
                it = w_pool.tile([P, K, 2], i32)
                nc.vector.tensor_copy(it[:], pv[:])
                fb = w_pool.tile([P, K, 2], f32)
                nc.vector.tensor_copy(fb[:], it[:])
                gt = w_pool.tile([P, K, 2], f32)
                nc.vector.tensor_tensor(out=gt[:], in0=fb[:], in1=pv[:], op=Alu.is_gt)
                f0 = w_pool.tile([P, K, 2], f32)
                nc.vector.tensor_tensor(out=f0[:], in0=fb[:], in1=gt[:], op=Alu.subtract)
                ab = w_pool.tile([P, K, 2], f32)
                nc.vector.tensor_tensor(out=ab[:], in0=pv[:], in1=f0[:], op=Alu.subtract)
                eq = w_pool.tile([P, K, 2], f32)
                nc.vector.tensor_scalar(eq[:], ab[:], 0.0, None, Alu.is_equal)
                abe = w_pool.tile([P, K, 2], f32)
                nc.vector.tensor_tensor(out=abe[:], in0=ab[:], in1=eq[:], op=Alu.add)

                # U2 entry id: (u0-1023)*1025 + (v0-1023)
                idf = w_pool.tile([P, K], f32)
                nc.vector.scalar_tensor_tensor(
                    out=idf[:], in0=f0[:, :, 0], scalar=1025.0, in1=f0[:, :, 1],
                    op0=Alu.mult, op1=Alu.add,
                )
                idf2 = w_pool.tile([P, K], f32)
                nc.vector.tensor_scalar(idf2[:], idf[:], 1049598.0, None, Alu.subtract)
                idx = w_pool.tile([P, K], i32)
                nc.vector.tensor_copy(idx[:], idf2[:])

                qq = q_pool.tile([P, K, 12], f32)
                for j in range(K):
                    g = nc.gpsimd.indirect_dma_start(
                        out=qq[:, j, :], out_offset=None, in_=u2_t.ap()[:],
                        in_offset=bass.IndirectOffsetOnAxis(ap=idx[:, j : j + 1], axis=0),
                    )
                    if j == 0:
                        add_dep_helper(g.ins, last_store.ins,
                                       reason="gathers read U2 after build")

                # qq layout per sample: [T00 T10 | T01 T11] (u-pairs adjacent)
                a_b = abe[:, :, 0:1].to_broadcast([P, K, 3])
                b_b = abe[:, :, 1:2].to_broadcast([P, K, 3])

                d0 = w_pool.tile([P, K, 3], f32)
                nc.vector.tensor_tensor(out=d0[:], in0=qq[:, :, 0:3], in1=qq[:, :, 3:6], op=Alu.subtract)
                dm0 = w_pool.tile([P, K, 3], f32)
                nc.vector.tensor_tensor(out=dm0[:], in0=d0[:], in1=a_b, op=Alu.mult)
                m0 = w_pool.tile([P, K, 3], f32)
                nc.vector.tensor_tensor(out=m0[:], in0=dm0[:], in1=qq[:, :, 3:6], op=Alu.add)

                d1 = w_pool.tile([P, K, 3], f32)
                nc.vector.tensor_tensor(out=d1[:], in0=qq[:, :, 6:9], in1=qq[:, :, 9:12], op=Alu.subtract)
                dm1 = w_pool.tile([P, K, 3], f32)
                nc.vector.tensor_tensor(out=dm1[:], in0=d1[:], in1=a_b, op=Alu.mult)
                m1 = w_pool.tile([P, K, 3], f32)
                nc.vector.tensor_tensor(out=m1[:], in0=dm1[:], in1=qq[:, :, 9:12], op=Alu.add)

                e = w_pool.tile([P, K, 3], f32)
                nc.vector.tensor_tensor(out=e[:], in0=m0[:], in1=m1[:], op=Alu.subtract)
                eb = w_pool.tile([P, K, 3], f32)
                nc.vector.tensor_tensor(out=eb[:], in0=e[:], in1=b_b, op=Alu.mult)
                ot = o_pool.tile([P, K, 3], f32)
                nc.vector.tensor_tensor(out=ot[:], in0=eb[:], in1=m1[:], op=Alu.add)

                nc.sync.dma_start(col_view[c], ot[:].rearrange("p k three -> p (k three)"))

    nc.compile()
    return nc


def kernel(uvs: np.ndarray, texture: np.ndarray) -> np.ndarray:
    from concourse import bass_utils

    if "nc" not in _cached:
        _cached["nc"] = _build()
    nc = _cached["nc"]

    tex_flat = np.ascontiguousarray(texture.reshape(W * W, 3), dtype=np.float32)
    uvs = np.ascontiguousarray(uvs, dtype=np.float32)
    in_maps = [
        {"texture": tex_flat, "uvs": uvs[g * NPC : (g + 1) * NPC]}
        for g in range(N_CORES)
    ]
    res = bass_utils.run_bass_kernel_spmd(
        nc, in_maps, core_ids=list(range(N_CORES)),
        trace=bool(int(os.environ.get("DIFFTEX_TRACE", "0"))),
    )
    _cached["last_results"] = res
    out = np.concatenate([r["colors"] for r in res.results], axis=0)
    return out



# revision 3
# speedup vs baseline: 1.0081x; 1.0081x over previous
"""v6: v4 gather structure (4096 x [128,1]-index indirect DMAs, the HW-proven
config) with the Pool engine kept saturated:

- All 32 chunks' gather indices + lerp weights are computed up front
  (phase A), overlapped with the U2 build, so gathers never wait on DVE.
- U2 build loads alternate between the sync and scalar HWDGE queues.
- Gathers then issue back-to-back; combines/stores trail behind.
"""

import os
import numpy as np

P = 128
N_CORES = 8
N = 4194304
NPC = N // N_CORES
W = 2048
K = 128
U2_ROWS = 1024
U2_COLS = 1025

_cached = {}


def _build(npc=NPC, k=K, n_cores=N_CORES):
    import concourse.bass as bass
    import concourse.tile as tile
    from concourse import bacc, mybir
    from concourse.tile import add_dep_helper
    from contextlib import ExitStack

    f32 = mybir.dt.float32
    i32 = mybir.dt.int32
    Alu = mybir.AluOpType
    K = k
    n_chunks = npc // (P * K)
    assert n_chunks * P * K == npc

    nc = bacc.Bacc(
        "TRN2",
        target_bir_lowering=False,
        debug=False,
        enable_asserts=False,
        num_devices=n_cores,
    )
    tex_t = nc.dram_tensor("texture", [W * W, 3], f32, kind="ExternalInput")
    uv_t = nc.dram_tensor("uvs", [npc, 2], f32, kind="ExternalInput")
    col_t = nc.dram_tensor("colors", [npc, 3], f32, kind="ExternalOutput")
    u2_t = nc.dram_tensor("u2tab", [U2_ROWS * U2_COLS, 6], f32, kind="Internal")

    tex_rows = tex_t.ap().rearrange("(r c) s -> r (c s)", c=W)   # [2048, 6144]
    u2_rows = u2_t.ap().rearrange("(r v) s -> r (v s)", v=U2_COLS)  # [1024, 6150]
    uv_view = uv_t.ap().rearrange("(c p k) two -> c p (k two)", p=P, k=K)
    col_view = col_t.ap().rearrange("(c p k) three -> c p (k three)", p=P, k=K)

    with tile.TileContext(nc) as tc:
        with ExitStack() as octx:
            idx_pool = octx.enter_context(tc.tile_pool(name="idx", bufs=1))
            idx_all = idx_pool.tile([P, n_chunks, K], i32)
            abe_all = idx_pool.tile([P, n_chunks, K, 2], f32)

            last_store = None
            with ExitStack() as ctx:
                b_pool = ctx.enter_context(tc.tile_pool(name="build", bufs=2))
                uv_pool = ctx.enter_context(tc.tile_pool(name="uv", bufs=3))
                w_pool = ctx.enter_context(tc.tile_pool(name="work", bufs=2))

                # ---- build U2 (8 blocks of 128 rows), loads split sync/scalar ----
                for b in range(U2_ROWS // P):
                    r0 = b * P
                    ta = b_pool.tile([P, 3 * U2_COLS], f32)
                    nc.sync.dma_start(
                        ta[:], tex_rows[1023 + r0 : 1023 + r0 + P, 3069 : 3069 + 3 * U2_COLS]
                    )
                    tb = b_pool.tile([P, 3 * U2_COLS], f32)
                    nc.scalar.dma_start(
                        tb[:], tex_rows[1024 + r0 : 1024 + r0 + P, 3069 : 3069 + 3 * U2_COLS]
                    )
                    ub = b_pool.tile([P, U2_COLS, 6], f32)
                    nc.vector.tensor_copy(
                        ub[:, :, 0:3], ta[:].rearrange("p (v s) -> p v s", s=3)
                    )
                    nc.vector.tensor_copy(
                        ub[:, :, 3:6], tb[:].rearrange("p (v s) -> p v s", s=3)
                    )
                    last_store = nc.sync.dma_start(
                        u2_rows[r0 : r0 + P, :], ub[:].rearrange("p v s -> p (v s)")
                    )

                # ---- phase A: indices + weights for every chunk ----
                for c in range(n_chunks):
                    uv = uv_pool.tile([P, K, 2], f32)
                    nc.sync.dma_start(uv[:].rearrange("p k two -> p (k two)"), uv_view[c])

                    pv = w_pool.tile([P, K, 2], f32)
                    nc.vector.tensor_scalar(pv[:], uv[:], 1.0, 1023.5, Alu.add, Alu.mult)
                    it = w_pool.tile([P, K, 2], i32)
                    nc.vector.tensor_copy(it[:], pv[:])
                    fb = w_pool.tile([P, K, 2], f32)
                    nc.vector.tensor_copy(fb[:], it[:])
                    gt = w_pool.tile([P, K, 2], f32)
                    nc.vector.tensor_tensor(out=gt[:], in0=fb[:], in1=pv[:], op=Alu.is_gt)
                    f0 = w_pool.tile([P, K, 2], f32)
                    nc.vector.tensor_tensor(out=f0[:], in0=fb[:], in1=gt[:], op=Alu.subtract)
                    ab = w_pool.tile([P, K, 2], f32)
                    nc.vector.tensor_tensor(out=ab[:], in0=pv[:], in1=f0[:], op=Alu.subtract)
                    eq = w_pool.tile([P, K, 2], f32)
                    nc.vector.tensor_scalar(eq[:], ab[:], 0.0, None, Alu.is_equal)
                    nc.vector.tensor_tensor(out=abe_all[:, c], in0=ab[:], in1=eq[:], op=Alu.add)

                    # U2 entry id: (u0-1023)*1025 + (v0-1023)
                    idf = w_pool.tile([P, K], f32)
                    nc.vector.scalar_tensor_tensor(
                        out=idf[:], in0=f0[:, :, 0], scalar=1025.0, in1=f0[:, :, 1],
                        op0=Alu.mult, op1=Alu.add,
                    )
                    idf2 = w_pool.tile([P, K], f32)
                    nc.vector.tensor_scalar(idf2[:], idf[:], 1049598.0, None, Alu.subtract)
                    nc.vector.tensor_copy(idx_all[:, c], idf2[:])

            # ---- phase B: gathers back-to-back, combines trailing ----
            with ExitStack() as ctx:
                q_pool = ctx.enter_context(tc.tile_pool(name="quads", bufs=4))
                c_pool = ctx.enter_context(tc.tile_pool(name="comb", bufs=2))
                o_pool = ctx.enter_context(tc.tile_pool(name="outs", bufs=4))

                for c in range(n_chunks):
                    qq = q_pool.tile([P, K, 12], f32)
                    for j in range(K):
                        g = nc.gpsimd.indirect_dma_start(
                            out=qq[:, j, :], out_offset=None, in_=u2_t.ap()[:],
                            in_offset=bass.IndirectOffsetOnAxis(
                                ap=idx_all[:, c, j : j + 1], axis=0
                            ),
                        )
                        if c == 0 and j == 0:
                            add_dep_helper(g.ins, last_store.ins,
                                           reason="gathers read U2 after build")

                    # qq layout per sample: [T00 T10 | T01 T11] (u-pairs adjacent)
                    a_b = abe_all[:, c, :, 0:1].to_broadcast([P, K, 3])
                    b_b = abe_all[:, c, :, 1:2].to_broadcast([P, K, 3])

                    d0 = c_pool.tile([P, K, 3], f32)
                    nc.vector.tensor_tensor(out=d0[:], in0=qq[:, :, 0:3], in1=qq[:, :, 3:6], op=Alu.subtract)
                    dm0 = c_pool.tile([P, K, 3], f32)
                    nc.vector.tensor_tensor(out=dm0[:], in0=d0[:], in1=a_b, op=Alu.mult)
                    m0 = c_pool.tile([P, K, 3], f32)
                    nc.vector.tensor_tensor(out=m0[:], in0=dm0[:], in1=qq[:, :, 3:6], op=Alu.add)

                    d1 = c_pool.tile([P, K, 3], f32)
                    nc.vector.tensor_tensor(out=d1[:], in0=qq[:, :, 6:9], in1=qq[:, :, 9:12], op=Alu.subtract)
                    dm1 = c_pool.tile([P, K, 3], f32)
                    nc.vector.tensor_tensor(out=dm1[:], in0=d1[:], in1=a_b, op=Alu.mult)
                    m1 = c_pool.tile([P, K, 3], f32)
                    nc.vector.tensor_tensor(out=m1[:], in0=dm1[:], in1=qq[:, :, 9:12], op=Alu.add)

                    e = c_pool.tile([P, K, 3], f32)
                    nc.vector.tensor_tensor(out=e[:], in0=m0[:], in1=m1[:], op=Alu.subtract)
                    eb = c_pool.tile([P, K, 3], f32)
                    nc.vector.tensor_tensor(out=eb[:], in0=e[:], in1=b_b, op=Alu.mult)
                    ot = o_pool.tile([P, K, 3], f32)
                    nc.vector.tensor_tensor(out=ot[:], in0=eb[:], in1=m1[:], op=Alu.add)

                    nc.sync.dma_start(col_view[c], ot[:].rearrange("p k three -> p (k three)"))

    nc.compile()
    return nc


def kernel(uvs: np.ndarray, texture: np.ndarray) -> np.ndarray:
    from concourse import bass_utils

    if "nc" not in _cached:
        _cached["nc"] = _build()
    nc = _cached["nc"]

    tex_flat = np.ascontiguousarray(texture.reshape(W * W, 3), dtype=np.float32)
    uvs = np.ascontiguousarray(uvs, dtype=np.float32)
    in_maps = [
        {"texture": tex_flat, "uvs": uvs[g * NPC : (g + 1) * NPC]}
        for g in range(N_CORES)
    ]
    res = bass_utils.run_bass_kernel_spmd(
        nc, in_maps, core_ids=list(range(N_CORES)),
        trace=bool(int(os.environ.get("DIFFTEX_TRACE", "0"))),
    )
    _cached["last_results"] = res
    out = np.concatenate([r["colors"] for r in res.results], axis=0)
    return out
